# revision 40
# baseline (speedup 1.0000x reference)
"""Trainium2 Bass kernel for nn_DeformableBlock (offset-conv -> deformable
conv v1 -> GroupNorm(32) -> ReLU), 8-core SPMD.

Sharding: core c -> (batch b = c//2, row-half h = c%2), rows [32h, 32h+32).
GroupNorm statistics are AllReduce'd across each (b,0)/(b,1) core pair.

v4 pipeline: offset conv folded into per-tile fp16 matmuls over the xz
window (zoff) with the 3x3 shifted-tap sum via a small DRAM round-trip;
z table in fp16 produced tap-major so gathers start early; gather indices
computed in the cheap plain layout and permuted into the SWDGE wrapped-16
layout with host-constant permutation matmuls; GN statistics accumulated
with PE matmuls (ones-vector contraction) + ACT squares so the DVE stays
on the bilinear accumulate; output staged in SBUF and written with one
contiguous fp16 store (host undoes the layout).

Per-core algorithm (z-first formulation):
  z_k = x . W_k (pointwise matmul per 3x3 tap) over a 40-row window, stored
  fp16 in DRAM as y-pair rows ypt[j] = (z[j], z[j+64]); one dma_gather per
  tap with OVERLAPPING 2KB elements (elem_size=1024, elem_step=512) reads
  ypt rows j and j+1 per index = all four bilinear corners in one
  descriptor, then fused scalar_tensor_tensor accumulate on the DVE.
"""
import functools
import numpy as np

import concourse.bass as bass
import concourse.bacc as bacc
import concourse.mybir as mybir
import concourse.tile as tile
from concourse.bass_utils import run_bass_kernel_spmd

F32 = mybir.dt.float32
FP16 = mybir.dt.float16
I16 = mybir.dt.int16
I32 = mybir.dt.int32
AOP = mybir.AluOpType
ACT = mybir.ActivationFunctionType

B, CIN, COUT, H, W = 4, 256, 256, 64, 64
K = 9
WROWS = 40            # z window rows (w0 = r0 - 4)
NPOS = 2048           # output positions per core (32 rows)
NWIN = WROWS * 64     # z window positions (2560)
NTW = NWIN // 128     # window tiles (20)
ZPAD = 72             # guard rows before the y-pair z table
NZROW = NWIN + 144    # 2704
NT = 16               # output position tiles of 128
EPS = 1e-5
GN_N = 2 * NPOS * 8   # elements per GN group (both cores of the pair)
J0 = 4 * 64           # window index of output position 0
# shift-matrix variants for the 3x3 shifted-tap sum: per tap, the main
# matrix (d=0) and, for nonzero shifts, the neighbor-tile boundary matrix
SSPEC = []
for _k in range(K):
    _s = 64 * (_k // 3 - 1) + (_k % 3 - 1)
    for _d in ([0] if _s == 0 else [0, 1 if _s > 0 else -1]):
        SSPEC.append((_k, _s, _d))


def build_program(use_cc=True):
    nc = bacc.Bacc(None, target_bir_lowering=False, num_devices=8,
                   num_swdge_queues=4)

    # ---------------- I/O ----------------
    xz_d = nc.dram_tensor("xz", [2, 128, NWIN], FP16, kind="ExternalInput")
    wdef_d = nc.dram_tensor("wdef", [2, 128, K, COUT], FP16, kind="ExternalInput")
    woffz_d = nc.dram_tensor("woffz", [2, 128, K, 18], FP16, kind="ExternalInput")
    bxy_d = nc.dram_tensor("bxy", [128, NT, 18], F32, kind="ExternalInput")
    pmat_d = nc.dram_tensor("pmat", [8, 128, 128], F32, kind="ExternalInput")
    ident_d = nc.dram_tensor("ident", [128, 128], F32, kind="ExternalInput")
    smat_d = nc.dram_tensor("smat", [len(SSPEC), 128, 128], FP16,
                            kind="ExternalInput")
    out_d = nc.dram_tensor("out", [128, NT * COUT], FP16, kind="ExternalOutput")

    with tile.TileContext(nc) as tc:
        with (
            tc.tile_pool(name="const", bufs=1) as cpool,
            tc.tile_pool(name="wm", bufs=1) as wmpool,
            tc.tile_pool(name="zst", bufs=2) as zstpool,
            tc.tile_pool(name="g", bufs=6) as gpool,
            tc.tile_pool(name="acc", bufs=1) as accpool,
            tc.tile_pool(name="outp", bufs=2) as outpool,
            tc.tile_pool(name="sh", bufs=2) as shpool,
            tc.tile_pool(name="ps", bufs=2, space="PSUM") as pspool,
            tc.tile_pool(name="ps2", bufs=1, space="PSUM") as ps2pool,
            tc.tile_pool(name="dram", bufs=1, space="DRAM") as dpool,
        ):
            # ------------- load inputs (sync queue; order = need time) ----
            xz = cpool.tile([128, 2, NWIN], FP16, tag="xz", name="xz")
            nc.sync.dma_start(xz[:], xz_d[:].rearrange("c p w -> p c w"))
            woffz = cpool.tile([128, 2, K, 18], FP16, tag="woffz", name="woffz")
            nc.sync.dma_start(
                woffz[:].rearrange("p c k o -> p c (k o)"),
                woffz_d[:].rearrange("c p k o -> p c (k o)"))
            bxy = cpool.tile([128, NT, 18], F32, tag="bxy", name="bxy")
            nc.sync.dma_start(bxy[:], bxy_d[:])
            # sc: col0 = bmask0, col1 = bmask1, col2 = wconst, col3 = ones
            sc_d = nc.dram_tensor("sc", [128, 4], F32, kind="ExternalInput")
            sc = cpool.tile([128, 4], F32, tag="sc", name="sc")
            nc.sync.dma_start(sc[:], sc_d[:])
            bmask = sc
            wconst = sc[:, 2:3]
            onescol = sc[:, 3:4]
            wdef = cpool.tile([128, 2, K, COUT], FP16, tag="wdef", name="wdef")
            nc.sync.dma_start(
                wdef[:].rearrange("p c k o -> p c (k o)"),
                wdef_d[:].rearrange("c p k o -> p c (k o)"))
            pmat = cpool.tile([128, 8, 128], F32, tag="pmat", name="pmat")
            nc.sync.dma_start(pmat[:], pmat_d[:].rearrange("u p m -> p u m"))
            ident = cpool.tile([128, 128], F32, tag="ident", name="ident")
            nc.sync.dma_start(ident[:], ident_d[:])
            smat = cpool.tile([128, len(SSPEC), 128], FP16, tag="smat",
                              name="smat")
            nc.sync.dma_start(smat[:], smat_d[:].rearrange("n p m -> p n m"))
            # rowc: [0:128] = onesrow, [128:640] = gnab
            rowc_d = nc.dram_tensor("rowc", [1, 640], F32, kind="ExternalInput")
            rowc = cpool.tile([1, 640], F32, tag="rowc", name="rowc")
            nc.sync.dma_start(rowc[:], rowc_d[:])
            onesrow = rowc[:, 0:128]
            gnab = rowc[:, 128:640]

            zbufs = [dpool.tile([NZROW, 2 * COUT], FP16, tag=f"zbuf{k}",
                                name=f"zbuf{k}") for k in range(K)]
            ccin = dpool.tile([1, 64], F32, tag="ccin", name="ccin")
            ccout = dpool.tile([1, 64], F32, tag="ccout", name="ccout")
            ccw = dpool.tile([1, 64], F32, tag="ccw", name="ccw")
            ccwo = dpool.tile([1, 64], F32, tag="ccwo", name="ccwo")

            # zero guard/boundary rows of every tap's zquad table; stores
            # overwrite the live slots afterwards; stale quad slots stay 0.
            zguard = cpool.tile([128, 2 * COUT], FP16, tag="zg", name="zg")
            nc.vector.memset(zguard[:], 0)
            gb0 = ZPAD + NWIN - 64    # 2568: first row with no s=1 write

            def zero_guard(k, engs):
                zb = zbufs[k][:]
                for i, (base, nrow) in enumerate(((60, 12), (gb0, 80))):
                    wr = bass.AP(
                        zb.tensor, zb.offset + base * 2 * COUT,
                        [[2 * COUT, nrow], [1, 2 * COUT]])
                    engs[i].dma_start(wr, zguard[0:nrow, :])

            # warm up the collective engine so the real AllReduce at the
            # tail doesn't pay ring-setup latency
            zgf = zguard[:].bitcast(F32)
            nc.scalar.dma_start(ccw[:], zgf[0:1, 0:64])
            if use_cc:
                nc.gpsimd.collective_compute(
                    "AllReduce", AOP.add,
                    replica_groups=[[0, 1], [2, 3], [4, 5], [6, 7]],
                    ins=[ccw[:].opt()], outs=[ccwo[:].opt()],
                )

            # ---------------- zoff: per-window-tile offset contributions ----
            # zoff[j, k, :] = sum_cin x[cin, j] * woff[cin, k, :]
            woffz_f = [woffz[:, ci].rearrange("p k c -> p (k c)")
                       for ci in range(2)]
            zoffsb = cpool.tile([128, 18, K * 18], FP16, tag="zoffsb",
                                name="zoffsb")
            for ti in range(18):  # window tiles 1..18 cover J0 +/- 65
                tt = 1 + ti
                zops = ps2pool.tile([128, K * 18], F32, bufs=2, tag="zops",
                                    name="zops")
                nc.tensor.matmul(zops[:], xz[:, 0, 128 * tt:128 * (tt + 1)],
                                 woffz_f[0], start=True, stop=False)
                nc.tensor.matmul(zops[:], xz[:, 1, 128 * tt:128 * (tt + 1)],
                                 woffz_f[1], start=False, stop=True)
                nc.vector.tensor_copy(zoffsb[:, ti, :], zops[:])

            # ---------------- z matmuls + store fp16 y-pair rows ----------
            # hoisted taps keep the sync queue free for the sh loads, so
            # their guards and both pair stores go via the scalar queue
            def z_tap(k, hoisted):
                zero_guard(k, (nc.scalar, nc.scalar) if hoisted
                           else (nc.sync, nc.scalar))
                zst = zstpool.tile([128, NTW, COUT], FP16, tag="zst",
                                   name=f"zst{k}")
                for tt in range(NTW):
                    zps = pspool.tile([128, COUT], F32, tag="zps", name="zps")
                    nc.tensor.matmul(zps[:], xz[:, 0, 128 * tt:128 * (tt + 1)],
                                     wdef[:, 0, k, :], start=True, stop=False)
                    nc.tensor.matmul(zps[:], xz[:, 1, 128 * tt:128 * (tt + 1)],
                                     wdef[:, 1, k, :], start=False, stop=True)
                    nc.scalar.copy(zst[:, tt, :], zps[:])
                zb = zbufs[k][:]
                engs = ((nc.scalar, nc.scalar) if hoisted
                        else (nc.sync, nc.scalar))
                for s, dlt in enumerate((0, 64)):
                    wrS = bass.AP(
                        zb.tensor,
                        zb.offset + (ZPAD - dlt) * 2 * COUT + s * COUT,
                        [[2 * COUT, 128], [128 * 2 * COUT, NTW], [1, COUT]])
                    engs[s].dma_start(wrS, zst[:])

            z_tap(0, hoisted=True)

            # --------- offsets: shift-matrix matmuls, summed in PSUM ------
            # off(p)[t] = sum_k zoff[J0 + 128t + q + s_k, k, :] via
            # host-constant shift matrices (border zero-pad folded in)
            pxy = cpool.tile([128, NT, 18], F32, tag="pxy", name="pxy")

            def off_half(h):
                for t in range(8 * h, 8 * h + 8):
                    offp = ps2pool.tile([128, 18], F32, bufs=2, tag="zops",
                                        name="offp")
                    for mi, (k, s, d) in enumerate(SSPEC):
                        nc.tensor.matmul(
                            offp[:], smat[:, mi, :],
                            zoffsb[:, 1 + t + d, 18 * k:18 * k + 18],
                            start=(mi == 0), stop=(mi == len(SSPEC) - 1))
                    nc.vector.tensor_add(pxy[:, t, :], offp[:], bxy[:, t, :])

            # ---------------- bilinear weights (plain layout, DVE) ------
            py_sl = pxy[:, :, 0:18:2]
            px_sl = pxy[:, :, 1:18:2]

            def wm(tag):
                return wmpool.tile([128, NT, K], F32, tag=tag, name=tag)

            def dev_floor_h(src, dst, h):
                sl = (slice(None), slice(8 * h, 8 * h + 8), slice(None))
                ii = wmpool.tile([128, NT, K], I32, tag="flr_i", name="fli")
                gt = wmpool.tile([128, NT, K], F32, tag="flr_g", name="flg")
                nc.vector.tensor_copy(ii[sl], src)       # fp32 -> int32
                nc.vector.tensor_copy(dst[sl], ii[sl])   # int32 -> fp32
                nc.vector.tensor_tensor(gt[sl], dst[sl], src, op=AOP.is_gt)
                nc.vector.tensor_tensor(dst[sl], dst[sl], gt[sl],
                                        op=AOP.subtract)

            y0 = wm("y0f")
            x0 = wm("x0f")
            idxp = wm("idxp")
            idxfr = wmpool.tile([128, K, 128], F32, tag="idxfr", name="idxfr")
            idx16 = wmpool.tile([128, K, 128], I16, tag="idx16", name="idx16")

            def idx_half(h):
                sl = (slice(None), slice(8 * h, 8 * h + 8), slice(None))
                dev_floor_h(py_sl[sl], y0, h)
                dev_floor_h(px_sl[sl], x0, h)
                # idx = clamp(y0 + wconst, 0, WROWS-1)*64 + (x0 - 16) + ZPAD
                rwp = wm("rwp")
                nc.vector.tensor_scalar_add(rwp[sl], y0[sl], wconst)
                nc.vector.tensor_scalar(rwp[sl], rwp[sl], 0.0,
                                        float(WROWS - 1),
                                        op0=AOP.max, op1=AOP.min)
                nc.vector.tensor_scalar(
                    rwp[sl], rwp[sl], 64.0, float(ZPAD - 16),
                    op0=AOP.mult, op1=AOP.add)
                nc.vector.tensor_tensor(idxp[sl], rwp[sl], x0[sl], op=AOP.add)
                # replicate into the SWDGE wrapped-16 layout via matmuls P_u:
                # idxfr[16a+v, k, 8t+u] = idxp(p = 128t + 16u + v, k)
                for u in range(8):
                    rps = ps2pool.tile([128, 8, K], F32, bufs=2, tag="zops",
                                       name="rps")
                    nc.tensor.matmul(
                        rps[:].rearrange("p t c -> p (t c)"), pmat[:, u, :],
                        idxp[sl].rearrange("p t c -> p (t c)"),
                        start=True, stop=True)
                    nc.vector.tensor_copy(
                        idxfr[:, :, 64 * h + u:64 * h + 64:8],
                        rps[:].rearrange("p t k -> p k t"))
                nc.vector.tensor_copy(
                    idx16[:, :, 64 * h:64 * h + 64],
                    idxfr[:, :, 64 * h:64 * h + 64])

            off_half(0)
            idx_half(0)
            z_tap(1, hoisted=True)
            off_half(1)
            idx_half(1)
            z_tap(2, hoisted=True)

            # ---------------- validity + corner weights -------------------
            ty = wm("ty"); tx = wm("tx")
            nc.vector.tensor_tensor(ty[:], py_sl, y0[:], op=AOP.subtract)
            nc.vector.tensor_tensor(tx[:], px_sl, x0[:], op=AOP.subtract)
            y1 = wm("y1"); x1 = wm("x1")
            nc.vector.tensor_scalar_add(y1[:], y0[:], 1.0)
            nc.vector.tensor_scalar_add(x1[:], x0[:], 1.0)
            vys = []
            for (yy, vtag) in ((y0, "0"), (y1, "1")):
                yg = wm("yg"); vy = wm("vy" + vtag)
                nc.vector.tensor_scalar(yg[:], yy[:], 16.0, 79.0,
                                        op0=AOP.max, op1=AOP.min)
                nc.vector.tensor_tensor(vy[:], yg[:], yy[:], op=AOP.is_equal)
                vys.append(vy)
            vxs = []
            for (xx, vtag) in ((x0, "0"), (x1, "1")):
                xg = wm("yg"); vx = wm("vx" + vtag)
                nc.vector.tensor_scalar(xg[:], xx[:], 16.0, 79.0,
                                        op0=AOP.max, op1=AOP.min)
                nc.vector.tensor_tensor(vx[:], xg[:], xx[:], op=AOP.is_equal)
                vxs.append(vx)

            omty = wm("omty"); omtx = wm("omtx")
            nc.vector.tensor_scalar(omty[:], ty[:], -1.0, 1.0,
                                    op0=AOP.mult, op1=AOP.add)
            nc.vector.tensor_scalar(omtx[:], tx[:], -1.0, 1.0,
                                    op0=AOP.mult, op1=AOP.add)
            wy = []
            for i, frac in enumerate((omty, ty)):
                wv = wm("wy" + str(i))
                nc.vector.tensor_tensor(wv[:], frac[:], vys[i][:], op=AOP.mult)
                wy.append(wv)
            wx = []
            for i, frac in enumerate((omtx, tx)):
                wv = wm("wx" + str(i))
                nc.vector.tensor_tensor(wv[:], frac[:], vxs[i][:], op=AOP.mult)
                wx.append(wv)

            # corner weights, laid out [128, kj, t] (kj = k*4 + 2*jy + jx)
            wgt_t = cpool.tile([128, 36, NT], F32, tag="wgt", name="wgt")
            for jy in range(2):
                for jx in range(2):
                    j = 2 * jy + jx
                    nc.vector.tensor_tensor(
                        wgt_t[:, j:36:4, :].rearrange("p k t -> p t k"),
                        wy[jy][:], wx[jx][:], op=AOP.mult)
            wgt16 = cpool.tile([128, 36, NT], FP16, tag="wgt16", name="wgt16")
            nc.vector.tensor_copy(
                wgt16[:].rearrange("p k t -> p (k t)"),
                wgt_t[:].rearrange("p k t -> p (k t)"))
            ones16 = cpool.tile([128, 1], FP16, tag="ones16", name="ones16")
            nc.vector.tensor_copy(ones16[:], onescol)

            # ------------- z matmuls + store fp16 y-pair rows (rest) ------
            for k in range(3, K):
                z_tap(k, hoisted=False)

            # ---------------- gather + weighted accumulate ----------------
            acc = accpool.tile([128, NT, COUT], FP16, tag="acc", name="acc")
            # GN stats: 4 PSUM accumulation groups in one bank:
            # cols (sum ch0-127, sum ch128-255, sq ch0-127, sq ch128-255)
            stps = ps2pool.tile([128, 4], F32, tag="stps", name="stps")
            nc.vector.memset(stps[:], 0)
            for k in range(K):
                zb = zbufs[k][:]
                zk = bass.AP(zb.tensor, zb.offset,
                             [[2 * COUT, NZROW - 1], [1, 4 * COUT]])
                for hh in range(4):
                    g = gpool.tile([128, 4, 4 * COUT], FP16, tag="g",
                                   name=f"g{k}_{hh}")
                    nc.gpsimd.dma_gather(
                        out_ap=g[:],
                        in_ap=zk,
                        idxs_ap=idx16[:, k, 32 * hh:32 * (hh + 1)],
                        num_idxs=NPOS // 4,
                        num_idxs_reg=NPOS // 4,
                        elem_size=4 * COUT,
                        elem_step=2 * COUT,
                        queue_num=hh,
                    )
                    for t in range(4 * hh, 4 * hh + 4):
                        tg = t - 4 * hh
                        # taps 3+: ACT (free of z-evac by then) computes the
                        # 4th corner's product; DVE folds it with a 2x add
                        act_help = k >= 3
                        act2 = k >= 6
                        ns = 2 if act2 else (3 if act_help else 4)
                        if act_help:
                            ctmp = outpool.tile([128, COUT], FP16, tag="ctmp",
                                                name="ctmp", bufs=3)
                            nc.scalar.activation(
                                ctmp[:], g[:, tg, 3 * COUT:4 * COUT],
                                ACT.Copy, scale=wgt_t[:, 4 * k + 3, t:t + 1])
                        if act2:
                            ctm2 = outpool.tile([128, COUT], FP16, tag="ctm2",
                                                name="ctm2", bufs=3)
                            nc.scalar.activation(
                                ctm2[:], g[:, tg, 2 * COUT:3 * COUT],
                                ACT.Copy, scale=wgt_t[:, 4 * k + 1, t:t + 1])
                        for s in range(ns):
                            j = (0, 2, 1, 3)[s]
                            first = (k == 0 and s == 0)
                            nc.vector.scalar_tensor_tensor(
                                acc[:, t, :],
                                g[:, tg, s * COUT:(s + 1) * COUT],
                                wgt16[:, 4 * k + j, t:t + 1],
                                g[:, tg, 0:COUT] if first else acc[:, t, :],
                                op0=AOP.mult,
                                op1=AOP.bypass if first else AOP.add)
                        if act_help:
                            nc.vector.tensor_tensor(
                                acc[:, t, :], acc[:, t, :], ctmp[:],
                                op=AOP.add)
                        if act2:
                            nc.vector.tensor_tensor(
                                acc[:, t, :], acc[:, t, :], ctm2[:],
                                op=AOP.add)
                        if k == K - 1:
                            # tile t is final: fold its GN stats in now
                            sqt = outpool.tile([128, COUT], FP16, tag="sqt",
                                               name="sqt")
                            nc.scalar.square(sqt[:], acc[:, t, :])
                            for c2 in range(2):
                                nc.tensor.matmul(
                                    stps[:, c2:c2 + 1],
                                    acc[:, t, 128 * c2:128 * (c2 + 1)],
                                    ones16[:],
                                    start=False, stop=(t == NT - 1),
                                    skip_group_check=True)
                                nc.tensor.matmul(
                                    stps[:, 2 + c2:3 + c2],
                                    sqt[:, 128 * c2:128 * (c2 + 1)],
                                    ones16[:],
                                    start=False, stop=(t == NT - 1),
                                    skip_group_check=True)

            # ---------------- GroupNorm stats + AllReduce ----------------
            # transpose [128,4] -> [4,128] via matmul with identity, then
            # reduce channel groups of 8 and DMA into the [1,64] CC row
            st4 = wmpool.tile([128, 4], F32, tag="st4", name="st4")
            nc.vector.tensor_copy(st4[:], stps[:])
            tps4 = ps2pool.tile([4, 128], F32, tag="tps4", name="tps4")
            nc.tensor.matmul(tps4[:], st4[:], ident[:], start=True, stop=True)
            red4 = wmpool.tile([4, 128], F32, tag="red4", name="red4")
            nc.vector.tensor_copy(red4[:], tps4[:])
            redg = wmpool.tile([4, 16], F32, tag="redg", name="redg")
            nc.vector.tensor_reduce(
                redg[:], red4[:].rearrange("p (g c) -> p g c", c=8),
                axis=mybir.AxisListType.X, op=AOP.add)
            ci_ap = ccin[:]
            nc.sync.dma_start(
                bass.AP(ci_ap.tensor, ci_ap.offset, [[16, 4], [1, 16]]),
                redg[:])
            if use_cc:
                nc.gpsimd.collective_compute(
                    "AllReduce", AOP.add,
                    replica_groups=[[0, 1], [2, 3], [4, 5], [6, 7]],
                    ins=[ccin[:].opt()], outs=[ccout[:].opt()],
                )
            else:
                nc.sync.dma_start(ccout[:], ccin[:])
            allst = wmpool.tile([1, 64], F32, tag="allst", name="allst")
            nc.sync.dma_start(allst[:], ccout[:])

            # mu = S/n; var = Q/n - mu^2; A = gamma*rstd; B = beta - mu*A
            mu = wmpool.tile([1, 32], F32, tag="mu", name="mu")
            var = wmpool.tile([1, 32], F32, tag="var", name="var")
            rstd = wmpool.tile([1, 32], F32, tag="rstd", name="rstd")
            nc.vector.tensor_scalar_mul(mu[:], allst[:, 0:32], 1.0 / GN_N)
            nc.vector.tensor_scalar_mul(var[:], allst[:, 32:64], 1.0 / GN_N)
            nc.vector.tensor_tensor(rstd[:], mu[:], mu[:], op=AOP.mult)
            nc.vector.tensor_tensor(var[:], var[:], rstd[:], op=AOP.subtract)
            nc.vector.tensor_scalar_add(var[:], var[:], EPS)
            nc.scalar.activation(rstd[:], var[:], ACT.Sqrt, bias=0.0)
            nc.vector.reciprocal(rstd[:], rstd[:])
            abrow = wmpool.tile([1, 512], F32, tag="abrow", name="abrow")
            rrep = wmpool.tile([1, 512], F32, tag="rrep", name="rrep")
            for c in range(8):
                nc.vector.tensor_copy(rrep[0:1, c:256:8], rstd[:])
                nc.vector.tensor_copy(rrep[0:1, 256 + c:512:8], mu[:])
            nc.vector.tensor_tensor(
                abrow[:, 0:256], rrep[:, 0:256], gnab[0:1, 0:256], op=AOP.mult)
            nc.vector.tensor_tensor(
                abrow[:, 256:512], rrep[:, 256:512], abrow[:, 0:256],
                op=AOP.mult)
            nc.vector.tensor_tensor(
                abrow[:, 256:512], gnab[0:1, 256:512], abrow[:, 256:512],
                op=AOP.subtract)
            abps = ps2pool.tile([128, 512], F32, tag="abps", name="abps")
            nc.tensor.matmul(abps[:], onesrow, abrow[:], start=True, stop=True)
            abbc = cpool.tile([128, 512], FP16, tag="abbc", name="abbc")
            nc.scalar.copy(abbc[:], abps[:])

            # ---------------- apply GN + ReLU, one contiguous store -------
            obuf = cpool.tile([128, NT, COUT], FP16, tag="obuf", name="obuf")
            for t in range(NT):
                ot = outpool.tile([128, COUT], FP16, tag="ot", name="ot")
                nc.vector.tensor_tensor(ot[:], acc[:, t, :], abbc[:, 0:256],
                                        op=AOP.mult)
                nc.vector.tensor_tensor(obuf[:, t, :], ot[:],
                                        abbc[:, 256:512], op=AOP.add)
            ofl = obuf[:].rearrange("p t c -> p (t c)")
            nc.scalar.activation(ofl, ofl, ACT.Relu)
            nc.sync.dma_start(out_d[:, :], ofl)

    nc.compile()
    return nc


@functools.lru_cache(maxsize=1)
def _program():
    return build_program()


def _prep_core(core, x, offw, offb, dw):
    b, h = core // 2, core % 2
    r0 = 32 * h
    w0 = r0 - 4

    xzarr = np.zeros((2, 128, WROWS, 64), np.float32)
    for i, r in enumerate(range(w0, w0 + WROWS)):
        if 0 <= r < H:
            xzarr[0, :, i, :] = x[b, 0:128, r, :]
            xzarr[1, :, i, :] = x[b, 128:256, r, :]

    # weights: wdef[ci, c, k, o] = dw[o, ci*128+c, ky, kx]
    dwr = dw.reshape(COUT, CIN, K).transpose(1, 2, 0)     # [cin, k, o]
    wdef = np.ascontiguousarray(dwr.reshape(2, 128, K, COUT))
    owr = offw.reshape(18, CIN, K).transpose(1, 2, 0)      # [cin, k, 18]
    woffz = np.ascontiguousarray(owr.reshape(2, 128, K, 18))

    pos = np.arange(NPOS)
    prow = r0 + pos // 64
    pcol = pos % 64
    ky = np.arange(K) // 3
    kx = np.arange(K) % 3
    # lifted (+16) base grids with offset bias folded in
    by = prow[:, None] - 1.0 + ky[None, :] + offb[0::2][None, :] + 16.0
    bx = pcol[:, None] - 1.0 + kx[None, :] + offb[1::2][None, :] + 16.0
    # plain layout: [NPOS, K] -> [128, NT, K] with position p at (p%128, p//128)
    byc = by.reshape(NT, 128, K).transpose(1, 0, 2)
    bxc = bx.reshape(NT, 128, K).transpose(1, 0, 2)
    bxy = np.empty((128, NT, 18), np.float32)
    bxy[:, :, 0::2] = byc
    bxy[:, :, 1::2] = bxc

    # sc: col0 = bmask (kx=0), col1 = bmask (kx=2), col2 = wconst, col3 = 1
    sc = np.ones((128, 4), np.float32)
    sc[0, 0] = sc[64, 0] = 0.0
    sc[63, 1] = sc[127, 1] = 0.0
    sc[:, 2] = float(-12 - r0)

    return {
        "xz": np.ascontiguousarray(
            xzarr.reshape(2, 128, NWIN)).astype(np.float16),
        "wdef": wdef.astype(np.float16), "woffz": woffz.astype(np.float16),
        "bxy": np.ascontiguousarray(bxy), "sc": sc,
    }


def kernel(x, offset_w, offset_b, deform_w, gn_gamma, gn_beta):
    x = np.asarray(x, np.float32)
    offw = np.asarray(offset_w, np.float32)
    offb = np.asarray(offset_b, np.float32)
    dw = np.asarray(deform_w, np.float32)
    gamma = np.asarray(gn_gamma, np.float32)
    beta = np.asarray(gn_beta, np.float32)

    nc = _program()

    ident = np.eye(128, dtype=np.float32)
    smat = np.zeros((len(SSPEC), 128, 128), np.float16)
    for mi, (k, s, d) in enumerate(SSPEC):
        kx = k % 3
        for q in range(128):
            if kx == 0 and q % 64 == 0:
                continue          # reference conv zero-pads x at col 0
            if kx == 2 and q % 64 == 63:
                continue          # and at col 63
            m = q + s - 128 * d
            if 0 <= m < 128:
                smat[mi, m, q] = 1.0
    rowc = np.concatenate(
        [np.ones(128, np.float32), gamma, beta]).reshape(1, 640)
    # pmat[u, q, m] = 1 iff q == 16u + (m % 16)
    pmat = np.zeros((8, 128, 128), np.float32)
    for u in range(8):
        for m in range(128):
            pmat[u, 16 * u + (m % 16), m] = 1.0

    in_maps = []
    for core in range(8):
        m = _prep_core(core, x, offw, offb, dw)
        m.update({"ident": ident, "rowc": rowc, "pmat": pmat,
                  "smat": smat})
        in_maps.append(m)

    global _last_in_maps
    _last_in_maps = in_maps

    res = run_bass_kernel_spmd(nc, in_maps, core_ids=list(range(8)))

    out = np.zeros((B, COUT, H, W), np.float32)
    for core in range(8):
        b, h = core // 2, core % 2
        o = np.asarray(res.results[core]["out"], np.float32)  # [128, NT*256]
        # obuf[part, t, c] = value at position p = 128t + part, channel c
        o = o.reshape(128, NT, COUT).transpose(1, 0, 2).reshape(NPOS, COUT)
        out[b, :, 32 * h:32 * h + 32, :] = (
            o.reshape(32, 64, COUT).transpose(2, 0, 1))
    return out


# revision 41
# speedup vs baseline: 1.0090x; 1.0090x over previous
"""Trainium2 Bass kernel for nn_DeformableBlock (offset-conv -> deformable
conv v1 -> GroupNorm(32) -> ReLU), 8-core SPMD.

Sharding: core c -> (batch b = c//2, row-half h = c%2), rows [32h, 32h+32).
GroupNorm statistics are AllReduce'd across each (b,0)/(b,1) core pair.

v4 pipeline: offset conv folded into per-tile fp16 matmuls over the xz
window (zoff) with the 3x3 shifted-tap sum via a small DRAM round-trip;
z table in fp16 produced tap-major so gathers start early; gather indices
computed in the cheap plain layout and permuted into the SWDGE wrapped-16
layout with host-constant permutation matmuls; GN statistics accumulated
with PE matmuls (ones-vector contraction) + ACT squares so the DVE stays
on the bilinear accumulate; output staged in SBUF and written with one
contiguous fp16 store (host undoes the layout).

Per-core algorithm (z-first formulation):
  z_k = x . W_k (pointwise matmul per 3x3 tap) over a 40-row window, stored
  fp16 in DRAM as y-pair rows ypt[j] = (z[j], z[j+64]); one dma_gather per
  tap with OVERLAPPING 2KB elements (elem_size=1024, elem_step=512) reads
  ypt rows j and j+1 per index = all four bilinear corners in one
  descriptor, then fused scalar_tensor_tensor accumulate on the DVE.
"""
import functools
import numpy as np

import concourse.bass as bass
import concourse.bacc as bacc
import concourse.mybir as mybir
import concourse.tile as tile
from concourse.bass_utils import run_bass_kernel_spmd

F32 = mybir.dt.float32
FP16 = mybir.dt.float16
I16 = mybir.dt.int16
I32 = mybir.dt.int32
AOP = mybir.AluOpType
ACT = mybir.ActivationFunctionType

B, CIN, COUT, H, W = 4, 256, 256, 64, 64
K = 9
WROWS = 40            # z window rows (w0 = r0 - 4)
NPOS = 2048           # output positions per core (32 rows)
NWIN = WROWS * 64     # z window positions (2560)
NTW = NWIN // 128     # window tiles (20)
ZPAD = 72             # guard rows before the y-pair z table
NZROW = NWIN + 144    # 2704
NT = 16               # output position tiles of 128
EPS = 1e-5
GN_N = 2 * NPOS * 8   # elements per GN group (both cores of the pair)
J0 = 4 * 64           # window index of output position 0
# shift-matrix variants for the 3x3 shifted-tap sum: per tap, the main
# matrix (d=0) and, for nonzero shifts, the neighbor-tile boundary matrix
SSPEC = []
for _k in range(K):
    _s = 64 * (_k // 3 - 1) + (_k % 3 - 1)
    for _d in ([0] if _s == 0 else [0, 1 if _s > 0 else -1]):
        SSPEC.append((_k, _s, _d))


def build_program(use_cc=True):
    nc = bacc.Bacc(None, target_bir_lowering=False, num_devices=8,
                   num_swdge_queues=4)

    # ---------------- I/O ----------------
    xz_d = nc.dram_tensor("xz", [2, 128, NWIN], FP16, kind="ExternalInput")
    wdef_d = nc.dram_tensor("wdef", [2, 128, K, COUT], FP16, kind="ExternalInput")
    woffz_d = nc.dram_tensor("woffz", [2, 128, K, 18], FP16, kind="ExternalInput")
    bxy_d = nc.dram_tensor("bxy", [128, NT, 18], F32, kind="ExternalInput")
    pmat_d = nc.dram_tensor("pmat", [8, 128, 128], F32, kind="ExternalInput")
    ident_d = nc.dram_tensor("ident", [128, 128], F32, kind="ExternalInput")
    smat_d = nc.dram_tensor("smat", [len(SSPEC), 128, 128], FP16,
                            kind="ExternalInput")
    out_d = nc.dram_tensor("out", [128, NT * COUT], FP16, kind="ExternalOutput")

    with tile.TileContext(nc) as tc:
        with (
            tc.tile_pool(name="const", bufs=1) as cpool,
            tc.tile_pool(name="wm", bufs=1) as wmpool,
            tc.tile_pool(name="zst", bufs=2) as zstpool,
            tc.tile_pool(name="g", bufs=6) as gpool,
            tc.tile_pool(name="acc", bufs=1) as accpool,
            tc.tile_pool(name="outp", bufs=2) as outpool,
            tc.tile_pool(name="sh", bufs=2) as shpool,
            tc.tile_pool(name="ps", bufs=2, space="PSUM") as pspool,
            tc.tile_pool(name="ps2", bufs=1, space="PSUM") as ps2pool,
            tc.tile_pool(name="dram", bufs=1, space="DRAM") as dpool,
        ):
            # ------------- load inputs (sync queue; order = need time) ----
            xz = cpool.tile([128, 2, NWIN], FP16, tag="xz", name="xz")
            nc.sync.dma_start(xz[:], xz_d[:].rearrange("c p w -> p c w"))
            woffz = cpool.tile([128, 2, K, 18], FP16, tag="woffz", name="woffz")
            nc.sync.dma_start(
                woffz[:].rearrange("p c k o -> p c (k o)"),
                woffz_d[:].rearrange("c p k o -> p c (k o)"))
            bxy = cpool.tile([128, NT, 18], F32, tag="bxy", name="bxy")
            nc.sync.dma_start(bxy[:], bxy_d[:])
            # sc: col0 = bmask0, col1 = bmask1, col2 = wconst, col3 = ones
            sc_d = nc.dram_tensor("sc", [128, 4], F32, kind="ExternalInput")
            sc = cpool.tile([128, 4], F32, tag="sc", name="sc")
            nc.sync.dma_start(sc[:], sc_d[:])
            bmask = sc
            wconst = sc[:, 2:3]
            onescol = sc[:, 3:4]
            wdef = cpool.tile([128, 2, K, COUT], FP16, tag="wdef", name="wdef")
            nc.sync.dma_start(
                wdef[:].rearrange("p c k o -> p c (k o)"),
                wdef_d[:].rearrange("c p k o -> p c (k o)"))
            pmat = cpool.tile([128, 8, 128], F32, tag="pmat", name="pmat")
            nc.sync.dma_start(pmat[:], pmat_d[:].rearrange("u p m -> p u m"))
            ident = cpool.tile([128, 128], F32, tag="ident", name="ident")
            nc.sync.dma_start(ident[:], ident_d[:])
            smat = cpool.tile([128, len(SSPEC), 128], FP16, tag="smat",
                              name="smat")
            nc.sync.dma_start(smat[:], smat_d[:].rearrange("n p m -> p n m"))
            # rowc: [0:128] = onesrow, [128:640] = gnab
            rowc_d = nc.dram_tensor("rowc", [1, 640], F32, kind="ExternalInput")
            rowc = cpool.tile([1, 640], F32, tag="rowc", name="rowc")
            nc.sync.dma_start(rowc[:], rowc_d[:])
            onesrow = rowc[:, 0:128]
            gnab = rowc[:, 128:640]

            zbufs = [dpool.tile([NZROW, 2 * COUT], FP16, tag=f"zbuf{k}",
                                name=f"zbuf{k}") for k in range(K)]
            ccin = dpool.tile([1, 64], F32, tag="ccin", name="ccin")
            ccout = dpool.tile([1, 64], F32, tag="ccout", name="ccout")
            ccw = dpool.tile([1, 64], F32, tag="ccw", name="ccw")
            ccwo = dpool.tile([1, 64], F32, tag="ccwo", name="ccwo")

            # zero guard/boundary rows of every tap's zquad table; stores
            # overwrite the live slots afterwards; stale quad slots stay 0.
            zguard = cpool.tile([128, 2 * COUT], FP16, tag="zg", name="zg")
            nc.vector.memset(zguard[:], 0)
            gb0 = ZPAD + NWIN - 64    # 2568: first row with no s=1 write

            def zero_guard(k, engs):
                zb = zbufs[k][:]
                for i, (base, nrow) in enumerate(((60, 12), (gb0, 80))):
                    wr = bass.AP(
                        zb.tensor, zb.offset + base * 2 * COUT,
                        [[2 * COUT, nrow], [1, 2 * COUT]])
                    engs[i].dma_start(wr, zguard[0:nrow, :])

            # warm up the collective engine so the real AllReduce at the
            # tail doesn't pay ring-setup latency
            zgf = zguard[:].bitcast(F32)
            nc.scalar.dma_start(ccw[:], zgf[0:1, 0:64])
            if use_cc:
                nc.gpsimd.collective_compute(
                    "AllReduce", AOP.add,
                    replica_groups=[[0, 1], [2, 3], [4, 5], [6, 7]],
                    ins=[ccw[:].opt()], outs=[ccwo[:].opt()],
                )

            # ---------------- zoff: per-window-tile offset contributions ----
            # zoff[j, k, :] = sum_cin x[cin, j] * woff[cin, k, :]
            woffz_f = [woffz[:, ci].rearrange("p k c -> p (k c)")
                       for ci in range(2)]
            zoffsb = cpool.tile([128, 18, K * 18], FP16, tag="zoffsb",
                                name="zoffsb")
            for ti in range(18):  # window tiles 1..18 cover J0 +/- 65
                tt = 1 + ti
                zops = ps2pool.tile([128, K * 18], F32, bufs=2, tag="zops",
                                    name="zops")
                nc.tensor.matmul(zops[:], xz[:, 0, 128 * tt:128 * (tt + 1)],
                                 woffz_f[0], start=True, stop=False)
                nc.tensor.matmul(zops[:], xz[:, 1, 128 * tt:128 * (tt + 1)],
                                 woffz_f[1], start=False, stop=True)
                nc.vector.tensor_copy(zoffsb[:, ti, :], zops[:])

            # ---------------- z matmuls + store fp16 y-pair rows ----------
            # hoisted taps keep the sync queue free for the sh loads, so
            # their guards and both pair stores go via the scalar queue
            def z_tap(k, hoisted):
                zero_guard(k, (nc.scalar, nc.scalar) if hoisted
                           else (nc.sync, nc.scalar))
                zst = zstpool.tile([128, NTW, COUT], FP16, tag="zst",
                                   name=f"zst{k}")
                for tt in range(NTW):
                    zps = pspool.tile([128, COUT], F32, tag="zps", name="zps")
                    nc.tensor.matmul(zps[:], xz[:, 0, 128 * tt:128 * (tt + 1)],
                                     wdef[:, 0, k, :], start=True, stop=False)
                    nc.tensor.matmul(zps[:], xz[:, 1, 128 * tt:128 * (tt + 1)],
                                     wdef[:, 1, k, :], start=False, stop=True)
                    nc.scalar.copy(zst[:, tt, :], zps[:])
                zb = zbufs[k][:]
                engs = ((nc.scalar, nc.scalar) if hoisted
                        else (nc.sync, nc.scalar))
                for s, dlt in enumerate((0, 64)):
                    wrS = bass.AP(
                        zb.tensor,
                        zb.offset + (ZPAD - dlt) * 2 * COUT + s * COUT,
                        [[2 * COUT, 128], [128 * 2 * COUT, NTW], [1, COUT]])
                    engs[s].dma_start(wrS, zst[:])

            z_tap(0, hoisted=True)

            # --------- offsets: shift-matrix matmuls, summed in PSUM ------
            # off(p)[t] = sum_k zoff[J0 + 128t + q + s_k, k, :] via
            # host-constant shift matrices (border zero-pad folded in)
            pxy = cpool.tile([128, NT, 18], F32, tag="pxy", name="pxy")

            def off_half(h):
                for t in range(8 * h, 8 * h + 8):
                    offp = ps2pool.tile([128, 18], F32, bufs=2, tag="zops",
                                        name="offp")
                    for mi, (k, s, d) in enumerate(SSPEC):
                        nc.tensor.matmul(
                            offp[:], smat[:, mi, :],
                            zoffsb[:, 1 + t + d, 18 * k:18 * k + 18],
                            start=(mi == 0), stop=(mi == len(SSPEC) - 1))
                    nc.vector.tensor_add(pxy[:, t, :], offp[:], bxy[:, t, :])

            # ---------------- bilinear weights (plain layout, DVE) ------
            py_sl = pxy[:, :, 0:18:2]
            px_sl = pxy[:, :, 1:18:2]

            def wm(tag):
                return wmpool.tile([128, NT, K], F32, tag=tag, name=tag)

            def dev_floor_h(src, dst, h):
                sl = (slice(None), slice(8 * h, 8 * h + 8), slice(None))
                ii = wmpool.tile([128, NT, K], I32, tag="flr_i", name="fli")
                gt = wmpool.tile([128, NT, K], F32, tag="flr_g", name="flg")
                nc.vector.tensor_copy(ii[sl], src)       # fp32 -> int32
                nc.vector.tensor_copy(dst[sl], ii[sl])   # int32 -> fp32
                nc.vector.tensor_tensor(gt[sl], dst[sl], src, op=AOP.is_gt)
                nc.vector.tensor_tensor(dst[sl], dst[sl], gt[sl],
                                        op=AOP.subtract)

            y0 = wm("y0f")
            x0 = wm("x0f")
            idxp = wm("idxp")
            idxfr = wmpool.tile([128, K, 128], F32, tag="idxfr", name="idxfr")
            idx16 = wmpool.tile([128, K, 128], I16, tag="idx16", name="idx16")

            def idx_half(h):
                sl = (slice(None), slice(8 * h, 8 * h + 8), slice(None))
                dev_floor_h(py_sl[sl], y0, h)
                dev_floor_h(px_sl[sl], x0, h)
                # idx = clamp(y0 + wconst, 0, WROWS-1)*64 + (x0 - 16) + ZPAD
                rwp = wm("rwp")
                nc.vector.tensor_scalar_add(rwp[sl], y0[sl], wconst)
                nc.vector.tensor_scalar(rwp[sl], rwp[sl], 0.0,
                                        float(WROWS - 1),
                                        op0=AOP.max, op1=AOP.min)
                nc.vector.tensor_scalar(
                    rwp[sl], rwp[sl], 64.0, float(ZPAD - 16),
                    op0=AOP.mult, op1=AOP.add)
                nc.vector.tensor_tensor(idxp[sl], rwp[sl], x0[sl], op=AOP.add)
                # replicate into the SWDGE wrapped-16 layout via matmuls P_u:
                # idxfr[16a+v, k, 8t+u] = idxp(p = 128t + 16u + v, k)
                for u in range(8):
                    rps = ps2pool.tile([128, 8, K], F32, bufs=2, tag="zops",
                                       name="rps")
                    nc.tensor.matmul(
                        rps[:].rearrange("p t c -> p (t c)"), pmat[:, u, :],
                        idxp[sl].rearrange("p t c -> p (t c)"),
                        start=True, stop=True)
                    nc.vector.tensor_copy(
                        idxfr[:, :, 64 * h + u:64 * h + 64:8],
                        rps[:].rearrange("p t k -> p k t"))
                nc.vector.tensor_copy(
                    idx16[:, :, 64 * h:64 * h + 64],
                    idxfr[:, :, 64 * h:64 * h + 64])

            off_half(0)
            idx_half(0)
            z_tap(1, hoisted=True)
            off_half(1)
            idx_half(1)
            z_tap(2, hoisted=True)

            # ---------------- validity + corner weights -------------------
            ty = wm("ty"); tx = wm("tx")
            nc.vector.tensor_tensor(ty[:], py_sl, y0[:], op=AOP.subtract)
            nc.vector.tensor_tensor(tx[:], px_sl, x0[:], op=AOP.subtract)
            y1 = wm("y1"); x1 = wm("x1")
            nc.vector.tensor_scalar_add(y1[:], y0[:], 1.0)
            nc.vector.tensor_scalar_add(x1[:], x0[:], 1.0)
            vys = []
            for (yy, vtag) in ((y0, "0"), (y1, "1")):
                yg = wm("yg"); vy = wm("vy" + vtag)
                nc.vector.tensor_scalar(yg[:], yy[:], 16.0, 79.0,
                                        op0=AOP.max, op1=AOP.min)
                nc.vector.tensor_tensor(vy[:], yg[:], yy[:], op=AOP.is_equal)
                vys.append(vy)
            vxs = []
            for (xx, vtag) in ((x0, "0"), (x1, "1")):
                xg = wm("yg"); vx = wm("vx" + vtag)
                nc.vector.tensor_scalar(xg[:], xx[:], 16.0, 79.0,
                                        op0=AOP.max, op1=AOP.min)
                nc.vector.tensor_tensor(vx[:], xg[:], xx[:], op=AOP.is_equal)
                vxs.append(vx)

            omty = wm("omty"); omtx = wm("omtx")
            nc.vector.tensor_scalar(omty[:], ty[:], -1.0, 1.0,
                                    op0=AOP.mult, op1=AOP.add)
            nc.vector.tensor_scalar(omtx[:], tx[:], -1.0, 1.0,
                                    op0=AOP.mult, op1=AOP.add)
            wy = []
            for i, frac in enumerate((omty, ty)):
                wv = wm("wy" + str(i))
                nc.vector.tensor_tensor(wv[:], frac[:], vys[i][:], op=AOP.mult)
                wy.append(wv)
            wx = []
            for i, frac in enumerate((omtx, tx)):
                wv = wm("wx" + str(i))
                nc.vector.tensor_tensor(wv[:], frac[:], vxs[i][:], op=AOP.mult)
                wx.append(wv)

            # corner weights, laid out [128, kj, t] (kj = k*4 + 2*jy + jx)
            wgt_t = cpool.tile([128, 36, NT], F32, tag="wgt", name="wgt")
            for jy in range(2):
                for jx in range(2):
                    j = 2 * jy + jx
                    nc.vector.tensor_tensor(
                        wgt_t[:, j:36:4, :].rearrange("p k t -> p t k"),
                        wy[jy][:], wx[jx][:], op=AOP.mult)
            wgt16 = cpool.tile([128, 36, NT], FP16, tag="wgt16", name="wgt16")
            nc.vector.tensor_copy(
                wgt16[:].rearrange("p k t -> p (k t)"),
                wgt_t[:].rearrange("p k t -> p (k t)"))
            ones16 = cpool.tile([128, 1], FP16, tag="ones16", name="ones16")
            nc.vector.tensor_copy(ones16[:], onescol)

            # ------------- z matmuls + store fp16 y-pair rows (rest) ------
            for k in range(3, K):
                z_tap(k, hoisted=False)

            # ---------------- gather + weighted accumulate ----------------
            acc = accpool.tile([128, NT, COUT], FP16, tag="acc", name="acc")
            # GN stats: 4 PSUM accumulation groups in one bank:
            # cols (sum ch0-127, sum ch128-255, sq ch0-127, sq ch128-255)
            stps = ps2pool.tile([128, 4], F32, tag="stps", name="stps")
            nc.vector.memset(stps[:], 0)
            for k in range(K):
                zb = zbufs[k][:]
                zk = bass.AP(zb.tensor, zb.offset,
                             [[2 * COUT, NZROW - 1], [1, 4 * COUT]])
                for hh in range(4):
                    g = gpool.tile([128, 4, 4 * COUT], FP16, tag="g",
                                   name=f"g{k}_{hh}")
                    nc.gpsimd.dma_gather(
                        out_ap=g[:],
                        in_ap=zk,
                        idxs_ap=idx16[:, k, 32 * hh:32 * (hh + 1)],
                        num_idxs=NPOS // 4,
                        num_idxs_reg=NPOS // 4,
                        elem_size=4 * COUT,
                        elem_step=2 * COUT,
                        queue_num=hh,
                    )
                    for t in range(4 * hh, 4 * hh + 4):
                        tg = t - 4 * hh
                        # taps 3+: ACT (free of z-evac by then) computes the
                        # 4th corner's product; DVE folds it with a 2x add
                        act_help = k >= 3
                        ns = 3 if act_help else 4
                        if act_help:
                            ctmp = outpool.tile([128, COUT], FP16, tag="ctmp",
                                                name="ctmp", bufs=3)
                            nc.scalar.activation(
                                ctmp[:], g[:, tg, 3 * COUT:4 * COUT],
                                ACT.Copy, scale=wgt_t[:, 4 * k + 3, t:t + 1])
                        for s in range(ns):
                            j = (0, 2, 1, 3)[s]
                            first = (k == 0 and s == 0)
                            nc.vector.scalar_tensor_tensor(
                                acc[:, t, :],
                                g[:, tg, s * COUT:(s + 1) * COUT],
                                wgt16[:, 4 * k + j, t:t + 1],
                                g[:, tg, 0:COUT] if first else acc[:, t, :],
                                op0=AOP.mult,
                                op1=AOP.bypass if first else AOP.add)
                        if act_help:
                            nc.vector.tensor_tensor(
                                acc[:, t, :], acc[:, t, :], ctmp[:],
                                op=AOP.add)
                        if k == K - 1:
                            # tile t is final: fold its GN stats in now
                            sqt = outpool.tile([128, COUT], FP16, tag="sqt",
                                               name="sqt")
                            nc.scalar.square(sqt[:], acc[:, t, :])
                            for c2 in range(2):
                                nc.tensor.matmul(
                                    stps[:, c2:c2 + 1],
                                    acc[:, t, 128 * c2:128 * (c2 + 1)],
                                    ones16[:],
                                    start=False, stop=(t == NT - 1),
                                    skip_group_check=True)
                                nc.tensor.matmul(
                                    stps[:, 2 + c2:3 + c2],
                                    sqt[:, 128 * c2:128 * (c2 + 1)],
                                    ones16[:],
                                    start=False, stop=(t == NT - 1),
                                    skip_group_check=True)

            # ---------------- GroupNorm stats + AllReduce ----------------
            # transpose [128,4] -> [4,128] via matmul with identity, then
            # reduce channel groups of 8 and DMA into the [1,64] CC row
            st4 = wmpool.tile([128, 4], F32, tag="st4", name="st4")
            nc.vector.tensor_copy(st4[:], stps[:])
            tps4 = ps2pool.tile([4, 128], F32, tag="tps4", name="tps4")
            nc.tensor.matmul(tps4[:], st4[:], ident[:], start=True, stop=True)
            red4 = wmpool.tile([4, 128], F32, tag="red4", name="red4")
            nc.vector.tensor_copy(red4[:], tps4[:])
            redg = wmpool.tile([4, 16], F32, tag="redg", name="redg")
            nc.vector.tensor_reduce(
                redg[:], red4[:].rearrange("p (g c) -> p g c", c=8),
                axis=mybir.AxisListType.X, op=AOP.add)
            ci_ap = ccin[:]
            nc.sync.dma_start(
                bass.AP(ci_ap.tensor, ci_ap.offset, [[16, 4], [1, 16]]),
                redg[:])
            if use_cc:
                nc.gpsimd.collective_compute(
                    "AllReduce", AOP.add,
                    replica_groups=[[0, 1], [2, 3], [4, 5], [6, 7]],
                    ins=[ccin[:].opt()], outs=[ccout[:].opt()],
                )
            else:
                nc.sync.dma_start(ccout[:], ccin[:])
            allst = wmpool.tile([1, 64], F32, tag="allst", name="allst")
            nc.sync.dma_start(allst[:], ccout[:])

            # mu = S/n; var = Q/n - mu^2; A = gamma*rstd; B = beta - mu*A
            mu = wmpool.tile([1, 32], F32, tag="mu", name="mu")
            var = wmpool.tile([1, 32], F32, tag="var", name="var")
            rstd = wmpool.tile([1, 32], F32, tag="rstd", name="rstd")
            nc.vector.tensor_scalar_mul(mu[:], allst[:, 0:32], 1.0 / GN_N)
            nc.vector.tensor_scalar_mul(var[:], allst[:, 32:64], 1.0 / GN_N)
            nc.vector.tensor_tensor(rstd[:], mu[:], mu[:], op=AOP.mult)
            nc.vector.tensor_tensor(var[:], var[:], rstd[:], op=AOP.subtract)
            nc.vector.tensor_scalar_add(var[:], var[:], EPS)
            nc.scalar.activation(rstd[:], var[:], ACT.Sqrt, bias=0.0)
            nc.vector.reciprocal(rstd[:], rstd[:])
            abrow = wmpool.tile([1, 512], F32, tag="abrow", name="abrow")
            rrep = wmpool.tile([1, 512], F32, tag="rrep", name="rrep")
            for c in range(8):
                nc.vector.tensor_copy(rrep[0:1, c:256:8], rstd[:])
                nc.vector.tensor_copy(rrep[0:1, 256 + c:512:8], mu[:])
            nc.vector.tensor_tensor(
                abrow[:, 0:256], rrep[:, 0:256], gnab[0:1, 0:256], op=AOP.mult)
            nc.vector.tensor_tensor(
                abrow[:, 256:512], rrep[:, 256:512], abrow[:, 0:256],
                op=AOP.mult)
            nc.vector.tensor_tensor(
                abrow[:, 256:512], gnab[0:1, 256:512], abrow[:, 256:512],
                op=AOP.subtract)
            abps = ps2pool.tile([128, 512], F32, tag="abps", name="abps")
            nc.tensor.matmul(abps[:], onesrow, abrow[:], start=True, stop=True)
            abbc = cpool.tile([128, 512], FP16, tag="abbc", name="abbc")
            nc.scalar.copy(abbc[:], abps[:])

            # ---------------- apply GN + ReLU, one contiguous store -------
            obuf = cpool.tile([128, NT, COUT], FP16, tag="obuf", name="obuf")
            for t in range(NT):
                ot = outpool.tile([128, COUT], FP16, tag="ot", name="ot")
                nc.vector.tensor_tensor(ot[:], acc[:, t, :], abbc[:, 0:256],
                                        op=AOP.mult)
                nc.vector.tensor_tensor(obuf[:, t, :], ot[:],
                                        abbc[:, 256:512], op=AOP.add)
            ofl = obuf[:].rearrange("p t c -> p (t c)")
            nc.scalar.activation(ofl, ofl, ACT.Relu)
            nc.sync.dma_start(out_d[:, :], ofl)

    nc.compile()
    return nc


@functools.lru_cache(maxsize=1)
def _program():
    return build_program()


def _prep_core(core, x, offw, offb, dw):
    b, h = core // 2, core % 2
    r0 = 32 * h
    w0 = r0 - 4

    xzarr = np.zeros((2, 128, WROWS, 64), np.float32)
    for i, r in enumerate(range(w0, w0 + WROWS)):
        if 0 <= r < H:
            xzarr[0, :, i, :] = x[b, 0:128, r, :]
            xzarr[1, :, i, :] = x[b, 128:256, r, :]

    # weights: wdef[ci, c, k, o] = dw[o, ci*128+c, ky, kx]
    dwr = dw.reshape(COUT, CIN, K).transpose(1, 2, 0)     # [cin, k, o]
    wdef = np.ascontiguousarray(dwr.reshape(2, 128, K, COUT))
    owr = offw.reshape(18, CIN, K).transpose(1, 2, 0)      # [cin, k, 18]
    woffz = np.ascontiguousarray(owr.reshape(2, 128, K, 18))

    pos = np.arange(NPOS)
    prow = r0 + pos // 64
    pcol = pos % 64
    ky = np.arange(K) // 3
    kx = np.arange(K) % 3
    # lifted (+16) base grids with offset bias folded in
    by = prow[:, None] - 1.0 + ky[None, :] + offb[0::2][None, :] + 16.0
    bx = pcol[:, None] - 1.0 + kx[None, :] + offb[1::2][None, :] + 16.0
    # plain layout: [NPOS, K] -> [128, NT, K] with position p at (p%128, p//128)
    byc = by.reshape(NT, 128, K).transpose(1, 0, 2)
    bxc = bx.reshape(NT, 128, K).transpose(1, 0, 2)
    bxy = np.empty((128, NT, 18), np.float32)
    bxy[:, :, 0::2] = byc
    bxy[:, :, 1::2] = bxc

    # sc: col0 = bmask (kx=0), col1 = bmask (kx=2), col2 = wconst, col3 = 1
    sc = np.ones((128, 4), np.float32)
    sc[0, 0] = sc[64, 0] = 0.0
    sc[63, 1] = sc[127, 1] = 0.0
    sc[:, 2] = float(-12 - r0)

    return {
        "xz": np.ascontiguousarray(
            xzarr.reshape(2, 128, NWIN)).astype(np.float16),
        "wdef": wdef.astype(np.float16), "woffz": woffz.astype(np.float16),
        "bxy": np.ascontiguousarray(bxy), "sc": sc,
    }


def kernel(x, offset_w, offset_b, deform_w, gn_gamma, gn_beta):
    x = np.asarray(x, np.float32)
    offw = np.asarray(offset_w, np.float32)
    offb = np.asarray(offset_b, np.float32)
    dw = np.asarray(deform_w, np.float32)
    gamma = np.asarray(gn_gamma, np.float32)
    beta = np.asarray(gn_beta, np.float32)

    nc = _program()

    ident = np.eye(128, dtype=np.float32)
    smat = np.zeros((len(SSPEC), 128, 128), np.float16)
    for mi, (k, s, d) in enumerate(SSPEC):
        kx = k % 3
        for q in range(128):
            if kx == 0 and q % 64 == 0:
                continue          # reference conv zero-pads x at col 0
            if kx == 2 and q % 64 == 63:
                continue          # and at col 63
            m = q + s - 128 * d
            if 0 <= m < 128:
                smat[mi, m, q] = 1.0
    rowc = np.concatenate(
        [np.ones(128, np.float32), gamma, beta]).reshape(1, 640)
    # pmat[u, q, m] = 1 iff q == 16u + (m % 16)
    pmat = np.zeros((8, 128, 128), np.float32)
    for u in range(8):
        for m in range(128):
            pmat[u, 16 * u + (m % 16), m] = 1.0

    in_maps = []
    for core in range(8):
        m = _prep_core(core, x, offw, offb, dw)
        m.update({"ident": ident, "rowc": rowc, "pmat": pmat,
                  "smat": smat})
        in_maps.append(m)

    global _last_in_maps
    _last_in_maps = in_maps

    res = run_bass_kernel_spmd(nc, in_maps, core_ids=list(range(8)))

    out = np.zeros((B, COUT, H, W), np.float32)
    for core in range(8):
        b, h = core // 2, core % 2
        o = np.asarray(res.results[core]["out"], np.float32)  # [128, NT*256]
        # obuf[part, t, c] = value at position p = 128t + part, channel c
        o = o.reshape(128, NT, COUT).transpose(1, 0, 2).reshape(NPOS, COUT)
        out[b, :, 32 * h:32 * h + 32, :] = (
            o.reshape(32, 64, COUT).transpose(2, 0, 1))
    return out
